# revision 7
# baseline (speedup 1.0000x reference)
"""MoE (noisy top-2 routing, 8 experts) on 8 Trainium2 NeuronCores.

Strategy (expert-parallel, per sharding hint):
  Host: gating network (tiny: 0.1% of FLOPs) + all-to-all dispatch —
      h = x@Wg+bg + noise*softplus(x@Wn+bn), exact top-2 + softmax,
      gather each expert's tokens with capacity factor 1.0 (1024
      tokens/expert); the ~1% overflow pairs are computed exactly on
      host in fp32.
  Device (single SPMD launch, one expert per core): per-expert FFN
      y = relu(x@W1+b1)@W2 + b2   on that expert's tokens (bf16
      matmuls, fp32 PSUM accumulation, weights fully SBUF-resident).
  Host: combine — scatter-add gate-weighted per-expert outputs.
"""
import sys

sys.path.insert(0, "/opt/trn_rl_repo")
import ml_dtypes
import numpy as np
import concourse.bass as bass  # noqa: F401
from concourse import bacc
import concourse.mybir as mybir
import concourse.tile as tile
from concourse.bass_utils import run_bass_kernel_spmd

N_CORES = 8
B, S, D, H, E = 2, 2048, 768, 3072, 8
T = B * S            # 4096 tokens
KD = D // 128        # 6 contraction chunks over D
NH = H // 128        # 24 h tiles
ND = D // 128        # 6 output d tiles
CAP = 1024           # per-expert token capacity (capacity factor 1.0)
NCH = 2              # token chunks
NC = CAP // NCH      # 512 tokens per chunk (= one full PSUM bank of fp32)

F32 = mybir.dt.float32
BF16 = mybir.dt.bfloat16
AF = mybir.ActivationFunctionType
BF16NP = ml_dtypes.bfloat16

_cache = {}
last_perf = {}


def _build_ffn():
    nc = bacc.Bacc("TRN2", target_bir_lowering=False, debug=False,
                   num_devices=N_CORES)
    # weight layouts are pre-packed on host so every DMA is row-contiguous:
    #   w1 col (hh*KD + k)*128 + c  = W1[k*128+p, hh*128+c]
    #   w2 col (dt*NH + hh)*128 + c = W2[hh*128+p, dt*128+c]
    #   xc col (ch*KD + k)*NC + t   = x_tok[k*128+p, ch*NC+t]
    w1 = nc.declare_dram_parameter("w1", [128, NH * KD * 128], BF16,
                                   isOutput=False)
    w2 = nc.declare_dram_parameter("w2", [128, ND * NH * 128], BF16,
                                   isOutput=False)
    b1 = nc.declare_dram_parameter("b1", [128, NH], F32, isOutput=False)
    b2 = nc.declare_dram_parameter("b2", [128, ND], F32, isOutput=False)
    xc = nc.declare_dram_parameter("xc", [128, NCH * KD * NC], BF16,
                                   isOutput=False)
    yT = nc.declare_dram_parameter("yT", [D, CAP], BF16, isOutput=True)

    with tile.TileContext(nc) as tc:
        with tc.tile_pool(name="sbig", bufs=1) as sbig, \
             tc.tile_pool(name="sout", bufs=4) as sout, \
             tc.tile_pool(name="psum", bufs=6, space="PSUM") as psum:
            x_sb = sbig.tile([128, NCH * KD * NC], BF16, tag="x")
            w1_sb = sbig.tile([128, NH * KD * 128], BF16, tag="w1")
            w2_sb = sbig.tile([128, ND * NH * 128], BF16, tag="w2")
            b1_sb = sbig.tile([128, NH], F32, tag="b1")
            b2_sb = sbig.tile([128, ND], F32, tag="b2")
            hid_sb = sbig.tile([128, NH * NCH * NC], BF16, tag="hid")
            XW = KD * NC
            W1W = KD * 128
            W2W = NH * 128

            def w1_dma(eng, lo, hi):
                eng.dma_start(out=w1_sb[:, lo * W1W:hi * W1W],
                              in_=w1[:, lo * W1W:hi * W1W])

            # Two HWDGE rings in parallel. Critical path: b1, w1[hh0] and
            # x[c0,k01] gate the first matmul group; interleave early w1
            # chunks with x pieces in consumption order, bulk on the other
            # ring.  sync ring:
            nc.sync.dma_start(out=b1_sb[:], in_=b1[:])
            w1_dma(nc.sync, 0, 1)
            nc.sync.dma_start(out=x_sb[:, 0:2 * NC], in_=xc[:, 0:2 * NC])
            w1_dma(nc.sync, 1, 2)
            nc.sync.dma_start(out=x_sb[:, 2 * NC:XW], in_=xc[:, 2 * NC:XW])
            w1_dma(nc.sync, 2, 4)
            nc.sync.dma_start(out=x_sb[:, XW:2 * XW], in_=xc[:, XW:2 * XW])
            w1_dma(nc.sync, 4, 8)
            nc.sync.dma_start(out=b2_sb[:], in_=b2[:])
            # scalar ring: w1 bulk, then w2 in dt-major consumption order
            w1_dma(nc.scalar, 8, 16)
            w1_dma(nc.scalar, 16, 24)
            for g in range(3):
                nc.scalar.dma_start(
                    out=w2_sb[:, g * 2 * W2W:(g + 1) * 2 * W2W],
                    in_=w2[:, g * 2 * W2W:(g + 1) * 2 * W2W])

            # ── layer 1: hid[hh, tok] = relu(sum_k w1[k,hh].T @ x[k, tok]) ──
            for hh in range(NH):
                pst = [psum.tile([128, NC], F32, tag="ps",
                                 name=f"ps1_{hh}_{c}") for c in range(NCH)]
                for k in range(KD):
                    lhs = w1_sb[:, (hh * KD + k) * 128:(hh * KD + k + 1) * 128]
                    for c in range(NCH):
                        nc.tensor.matmul(
                            out=pst[c][:], lhsT=lhs,
                            rhs=x_sb[:, (c * KD + k) * NC:
                                     (c * KD + k + 1) * NC],
                            start=(k == 0), stop=(k == KD - 1))
                for c in range(NCH):
                    nc.scalar.activation(
                        hid_sb[:, (hh * NCH + c) * NC:(hh * NCH + c + 1) * NC],
                        pst[c][:], AF.Relu, bias=b1_sb[:, hh:hh + 1])

            # ── layer 2: y[dt, tok] = sum_hh w2[hh,dt].T @ hid[hh, tok] ──
            for dt_ in range(ND):
                psy = [psum.tile([128, NC], F32, tag="ps",
                                 name=f"ps2_{dt_}_{c}") for c in range(NCH)]
                for hh in range(NH):
                    lhs = w2_sb[:, (dt_ * NH + hh) * 128:
                                (dt_ * NH + hh + 1) * 128]
                    for c in range(NCH):
                        nc.tensor.matmul(
                            out=psy[c][:], lhsT=lhs,
                            rhs=hid_sb[:, (hh * NCH + c) * NC:
                                       (hh * NCH + c + 1) * NC],
                            start=(hh == 0), stop=(hh == NH - 1))
                for c in range(NCH):
                    yo = sout.tile([128, NC], BF16, tag="yo",
                                   name=f"yo_{dt_}_{c}")
                    nc.vector.tensor_scalar_add(yo[:], psy[c][:],
                                                b2_sb[:, dt_:dt_ + 1])
                    nc.sync.dma_start(
                        out=yT[dt_ * 128:(dt_ + 1) * 128, c * NC:(c + 1) * NC],
                        in_=yo[:])
    nc.compile()
    return nc


def kernel(x, noise, Wg, bg, Wn, bn, W1, b1, W2, b2):
    x = np.asarray(x, dtype=np.float32)
    noise = np.asarray(noise, dtype=np.float32)
    Wg = np.asarray(Wg, dtype=np.float32)
    bg = np.asarray(bg, dtype=np.float32)
    Wn = np.asarray(Wn, dtype=np.float32)
    bn = np.asarray(bn, dtype=np.float32)
    W1 = np.asarray(W1, dtype=np.float32)
    b1 = np.asarray(b1, dtype=np.float32)
    W2 = np.asarray(W2, dtype=np.float32)
    b2 = np.asarray(b2, dtype=np.float32)

    if "ffn" not in _cache:
        _cache["ffn"] = _build_ffn()

    x2d = x.reshape(T, D)
    n2d = noise.reshape(T, E)

    # ── host gating: h = x@Wg+bg + noise*softplus(x@Wn+bn), exact top-2 ──
    gate = x2d @ Wg + bg
    hlog = gate + n2d * np.logaddexp(0.0, x2d @ Wn + bn)
    idx = np.argsort(-hlog, axis=1, kind="stable")[:, :2]     # [T, 2]
    vals = np.take_along_axis(hlog, idx, axis=1)
    q = np.exp(vals[:, 1] - vals[:, 0])
    p1 = 1.0 / (1.0 + q)
    probs = np.stack([p1, q * p1], axis=1).astype(np.float32)  # [T, 2]

    # ── host dispatch: gather tokens per expert (capacity CAP), pack ──
    xT = x2d.T                                                 # [D, T] view
    in_maps = []
    idxs, gates, spill = [], [], []
    for e in range(E):
        m = idx == e
        sel = np.nonzero(m.any(axis=1))[0]
        gv = np.where(m[sel, 0], probs[sel, 0], probs[sel, 1])
        if sel.size > CAP:                 # overflow pairs -> host fp32
            spill.append((e, sel[CAP:], gv[CAP:]))
            sel, gv = sel[:CAP], gv[:CAP]
        idxs.append(sel)
        gates.append(gv)
        xe = np.zeros((D, CAP), dtype=np.float32)
        xe[:, :sel.size] = xT[:, sel]
        # [k, p, ch, t] -> [p, ch, k, t]
        xp = np.ascontiguousarray(
            xe.reshape(KD, 128, NCH, NC).transpose(1, 2, 0, 3)
        ).reshape(128, NCH * KD * NC).astype(BF16NP)
        w1p = np.ascontiguousarray(
            W1[e].reshape(KD, 128, NH, 128).transpose(1, 2, 0, 3)
        ).reshape(128, NH * KD * 128).astype(BF16NP)
        w2p = np.ascontiguousarray(
            W2[e].reshape(NH, 128, ND, 128).transpose(1, 2, 0, 3)
        ).reshape(128, ND * NH * 128).astype(BF16NP)
        in_maps.append({
            "w1": w1p,
            "w2": w2p,
            "b1": np.ascontiguousarray(b1[e].reshape(NH, 128).T),
            "b2": np.ascontiguousarray(b2[e].reshape(ND, 128).T),
            "xc": xp,
        })

    res = run_bass_kernel_spmd(_cache["ffn"], in_maps,
                               core_ids=list(range(N_CORES)))
    last_perf["p2"] = res.exec_time_ns
    if res.instructions_and_trace:
        last_perf["p2_insts"] = res.instructions_and_trace[0]

    # ── host combine: gate-weighted scatter-add ──
    out = np.zeros((T, D), dtype=np.float32)
    for e in range(E):
        sel = idxs[e]
        yT_ = np.asarray(res.results[e]["yT"], dtype=np.float32)  # [D, CAP]
        out[sel] += yT_[:, :sel.size].T * gates[e][:, None]
    for e, sel, gv in spill:                                   # host overflow
        hid = np.maximum(x2d[sel] @ W1[e] + b1[e], 0.0)
        out[sel] += (hid @ W2[e] + b2[e]) * gv[:, None]
    return out.reshape(B, S, D)


# revision 12
# speedup vs baseline: 1.0038x; 1.0038x over previous
"""MoE (noisy top-2 routing, 8 experts) on 8 Trainium2 NeuronCores.

Strategy (expert-parallel, per sharding hint):
  Host: gating network (tiny: 0.1% of FLOPs) + all-to-all dispatch —
      h = x@Wg+bg + noise*softplus(x@Wn+bn), exact top-2 + softmax,
      gather each expert's tokens with capacity factor 1.0 (1024
      tokens/expert); the ~1% overflow pairs are computed exactly on
      host in fp32.
  Device (single SPMD launch, one expert per core): per-expert FFN
      y = relu(x@W1+b1)@W2 + b2   on that expert's tokens (bf16
      matmuls, fp32 PSUM accumulation, weights fully SBUF-resident).
  Host: combine — scatter-add gate-weighted per-expert outputs.
"""
import sys

sys.path.insert(0, "/opt/trn_rl_repo")
import ml_dtypes
import numpy as np
import concourse.bass as bass  # noqa: F401
from concourse import bacc
import concourse.mybir as mybir
import concourse.tile as tile
from concourse.bass_utils import run_bass_kernel_spmd

N_CORES = 8
B, S, D, H, E = 2, 2048, 768, 3072, 8
T = B * S            # 4096 tokens
KD = D // 128        # 6 contraction chunks over D
NH = H // 128        # 24 h tiles
ND = D // 128        # 6 output d tiles
CAP = 1024           # per-expert token capacity (capacity factor 1.0)
NCH = 2              # token chunks
NC = CAP // NCH      # 512 tokens per chunk (= one full PSUM bank of fp32)

F32 = mybir.dt.float32
BF16 = mybir.dt.bfloat16
AF = mybir.ActivationFunctionType
BF16NP = ml_dtypes.bfloat16

_cache = {}
last_perf = {}


def _build_ffn():
    nc = bacc.Bacc("TRN2", target_bir_lowering=False, debug=False,
                   num_devices=N_CORES)
    # weight layouts are pre-packed on host so every DMA is row-contiguous:
    #   w1 col (hh*KD + k)*128 + c  = W1[k*128+p, hh*128+c]
    #   w2 col (dt*NH + hh)*128 + c = W2[hh*128+p, dt*128+c]
    #   xc col (ch*KD + k)*NC + t   = x_tok[k*128+p, ch*NC+t]
    w1 = nc.declare_dram_parameter("w1", [128, NH * KD * 128], BF16,
                                   isOutput=False)
    w2 = nc.declare_dram_parameter("w2", [128, ND * NH * 128], BF16,
                                   isOutput=False)
    b1 = nc.declare_dram_parameter("b1", [128, NH], F32, isOutput=False)
    xc = nc.declare_dram_parameter("xc", [128, NCH * KD * NC], BF16,
                                   isOutput=False)
    yT = nc.declare_dram_parameter("yT", [D, CAP], BF16, isOutput=True)

    with tile.TileContext(nc) as tc:
        with tc.tile_pool(name="sbig", bufs=1) as sbig, \
             tc.tile_pool(name="sout", bufs=4) as sout, \
             tc.tile_pool(name="psum", bufs=6, space="PSUM") as psum:
            x_sb = sbig.tile([128, NCH * KD * NC], BF16, tag="x")
            w1_sb = sbig.tile([128, NH * KD * 128], BF16, tag="w1")
            w2_sb = sbig.tile([128, ND * NH * 128], BF16, tag="w2")
            b1_sb = sbig.tile([128, NH], F32, tag="b1")
            hid_sb = sbig.tile([128, NH * NCH * NC], BF16, tag="hid")
            XW = KD * NC
            W1W = KD * 128
            W2W = NH * 128

            def w1_dma(eng, lo, hi):
                eng.dma_start(out=w1_sb[:, lo * W1W:hi * W1W],
                              in_=w1[:, lo * W1W:hi * W1W])

            # Two HWDGE rings in parallel; each DMA pays ~0.6us issue +
            # ~2us completion latency, so order exactly by first use.
            # First matmul chain (hh=0, chunk 0) needs w1[hh0] + x[c0].
            w1_dma(nc.sync, 0, 1)
            nc.sync.dma_start(out=x_sb[:, 0:3 * NC], in_=xc[:, 0:3 * NC])
            nc.sync.dma_start(out=x_sb[:, 3 * NC:XW], in_=xc[:, 3 * NC:XW])
            nc.sync.dma_start(out=b1_sb[:], in_=b1[:])
            nc.sync.dma_start(out=x_sb[:, XW:2 * XW], in_=xc[:, XW:2 * XW])
            # scalar ring: w1 bulk, then w2 in dt-major consumption order
            w1_dma(nc.scalar, 1, 6)
            w1_dma(nc.scalar, 6, 12)
            w1_dma(nc.scalar, 12, 18)
            w1_dma(nc.scalar, 18, 24)
            for g in range(3):
                nc.scalar.dma_start(
                    out=w2_sb[:, g * 2 * W2W:(g + 1) * 2 * W2W],
                    in_=w2[:, g * 2 * W2W:(g + 1) * 2 * W2W])

            # ── layer 1: hid[hh, tok] = relu(sum_k w1[k,hh].T @ x[k, tok]) ──
            # chunk-outer so the first accumulation chain only needs x[c0]
            for hh in range(NH):
                for c in range(NCH):
                    pst = psum.tile([128, NC], F32, tag="ps",
                                    name=f"ps1_{hh}_{c}")
                    for k in range(KD):
                        nc.tensor.matmul(
                            out=pst[:],
                            lhsT=w1_sb[:, (hh * KD + k) * 128:
                                       (hh * KD + k + 1) * 128],
                            rhs=x_sb[:, (c * KD + k) * NC:
                                     (c * KD + k + 1) * NC],
                            start=(k == 0), stop=(k == KD - 1))
                    nc.scalar.activation(
                        hid_sb[:, (hh * NCH + c) * NC:(hh * NCH + c + 1) * NC],
                        pst[:], AF.Relu, bias=b1_sb[:, hh:hh + 1])

            # ── layer 2: y[dt, tok] = sum_hh w2[hh,dt].T @ hid[hh, tok] ──
            # (b2 is applied on the host; evictions are pure copies split
            # across the Vector and Scalar engines)
            for dt_ in range(ND):
                for c in range(NCH):
                    psy = psum.tile([128, NC], F32, tag="ps",
                                    name=f"ps2_{dt_}_{c}")
                    for hh in range(NH):
                        nc.tensor.matmul(
                            out=psy[:],
                            lhsT=w2_sb[:, (dt_ * NH + hh) * 128:
                                       (dt_ * NH + hh + 1) * 128],
                            rhs=hid_sb[:, (hh * NCH + c) * NC:
                                       (hh * NCH + c + 1) * NC],
                            start=(hh == 0), stop=(hh == NH - 1))
                    yo = sout.tile([128, NC], BF16, tag="yo",
                                   name=f"yo_{dt_}_{c}")
                    if c == 0:
                        nc.vector.tensor_copy(yo[:], psy[:])
                    else:
                        nc.scalar.copy(yo[:], psy[:])
                    nc.sync.dma_start(
                        out=yT[dt_ * 128:(dt_ + 1) * 128, c * NC:(c + 1) * NC],
                        in_=yo[:])
    nc.compile()
    return nc


def kernel(x, noise, Wg, bg, Wn, bn, W1, b1, W2, b2):
    x = np.asarray(x, dtype=np.float32)
    noise = np.asarray(noise, dtype=np.float32)
    Wg = np.asarray(Wg, dtype=np.float32)
    bg = np.asarray(bg, dtype=np.float32)
    Wn = np.asarray(Wn, dtype=np.float32)
    bn = np.asarray(bn, dtype=np.float32)
    W1 = np.asarray(W1, dtype=np.float32)
    b1 = np.asarray(b1, dtype=np.float32)
    W2 = np.asarray(W2, dtype=np.float32)
    b2 = np.asarray(b2, dtype=np.float32)

    if "ffn" not in _cache:
        _cache["ffn"] = _build_ffn()

    x2d = x.reshape(T, D)
    n2d = noise.reshape(T, E)

    # ── host gating: h = x@Wg+bg + noise*softplus(x@Wn+bn), exact top-2 ──
    gate = x2d @ Wg + bg
    hlog = gate + n2d * np.logaddexp(0.0, x2d @ Wn + bn)
    idx = np.argsort(-hlog, axis=1, kind="stable")[:, :2]     # [T, 2]
    vals = np.take_along_axis(hlog, idx, axis=1)
    q = np.exp(vals[:, 1] - vals[:, 0])
    p1 = 1.0 / (1.0 + q)
    probs = np.stack([p1, q * p1], axis=1).astype(np.float32)  # [T, 2]

    # ── host dispatch: gather tokens per expert (capacity CAP), pack ──
    xT = x2d.T                                                 # [D, T] view
    in_maps = []
    idxs, gates, spill = [], [], []
    for e in range(E):
        m = idx == e
        sel = np.nonzero(m.any(axis=1))[0]
        gv = np.where(m[sel, 0], probs[sel, 0], probs[sel, 1])
        if sel.size > CAP:                 # overflow pairs -> host fp32
            spill.append((e, sel[CAP:], gv[CAP:]))
            sel, gv = sel[:CAP], gv[:CAP]
        idxs.append(sel)
        gates.append(gv)
        xe = np.zeros((D, CAP), dtype=np.float32)
        xe[:, :sel.size] = xT[:, sel]
        # [k, p, ch, t] -> [p, ch, k, t]
        xp = np.ascontiguousarray(
            xe.reshape(KD, 128, NCH, NC).transpose(1, 2, 0, 3)
        ).reshape(128, NCH * KD * NC).astype(BF16NP)
        w1p = np.ascontiguousarray(
            W1[e].reshape(KD, 128, NH, 128).transpose(1, 2, 0, 3)
        ).reshape(128, NH * KD * 128).astype(BF16NP)
        w2p = np.ascontiguousarray(
            W2[e].reshape(NH, 128, ND, 128).transpose(1, 2, 0, 3)
        ).reshape(128, ND * NH * 128).astype(BF16NP)
        in_maps.append({
            "w1": w1p,
            "w2": w2p,
            "b1": np.ascontiguousarray(b1[e].reshape(NH, 128).T),
            "xc": xp,
        })

    res = run_bass_kernel_spmd(_cache["ffn"], in_maps,
                               core_ids=list(range(N_CORES)))
    last_perf["p2"] = res.exec_time_ns
    if res.instructions_and_trace:
        last_perf["p2_insts"] = res.instructions_and_trace[0]

    # ── host combine: gate-weighted scatter-add ──
    out = np.zeros((T, D), dtype=np.float32)
    for e in range(E):
        sel = idxs[e]
        yT_ = np.asarray(res.results[e]["yT"], dtype=np.float32)  # [D, CAP]
        out[sel] += (yT_[:, :sel.size].T + b2[e]) * gates[e][:, None]
    for e, sel, gv in spill:                                   # host overflow
        hid = np.maximum(x2d[sel] @ W1[e] + b1[e], 0.0)
        out[sel] += (hid @ W2[e] + b2[e]) * gv[:, None]
    return out.reshape(B, S, D)
